# revision 10
# baseline (speedup 1.0000x reference)
"""AFNO spectral attention kernel for 8 Trainium2 NeuronCores.

Math reduction (identical to the verified baseline):
  rfft2 -> first-8-W-mode block-diag channel matmul (x sigmoid(gate)) ->
  irfft2 -> residual -> out projection collapses to (H-axis FFTs cancel):

    xlp = x @ M2           M2 = w-axis low-pass (64x64, symmetric), per row
    xs  = (x - xlp) + xlp @ A       A = blockdiag(sigmoid(g_b) * W_b)
    y   = xs + xs @ W'              W' = rescale * W_out^T

Engine mapping per core (2 of 16 images, data-parallel over batch):
  Full-magnitude paths run fp16 on the PE; the small-coefficient channel
  matmuls (A ~0.01, W' ~0.002 entries) run fp8e4m3 with
  MatmulPerfMode.DoubleRow (two 128-channel k-tiles per matmul, 2x MACs).
  DR moving operands must be contiguous [K, 2, N] blocks (strided inner
  dims silently corrupt the second k-tile), so adr/w8c are stored
  pre-blocked per band / (chunk-pair, column-split).

  The single psum accumulator py carries 32*xs then 32*y:
    py[j]  = (32*Mh2)^T @ xn[j] + (xlp/4) @ (128*A)     [= 32*xs]
    py[j] += (xs/4) @ (128*W')                          [= 32*y]
  Power-of-2 scales keep every fp8 operand in e4m3's normal range (TRN
  flushes fp8 subnormals) and cancel exactly in the fp32 psum; the final
  1/32 rides the one mandatory psum->SBUF evacuation.

  Per group of 512 rows (4 subtiles of 128 rows = 2 h-rows x 64 w):
    xlpT8     = (xn[j][:,k]^T @ M2)/4       fp16 PE transpose, ACT evac fp8
    py[j]     = hp + z  (2 fp16 + 6 DR + 2 fp8 matmuls)
    xs8[j]    = py[j]/128                   DVE evac -> fp8 row-space xs/4
    xsT8[:,j] = xs8[j][:,k]^T @ I8          fp8 PE transposes, ACT evac
    py[j]    += proj (9 DR matmuls)
    y16[j]    = py[j]/32                    DVE evac -> fp16, DMA out
"""

import numpy as np

import concourse.mybir as mybir
import concourse.tile as tile
from concourse import bacc
from concourse.bass_utils import run_bass_kernel_spmd

B, N_TOK, C = 16, 4096, 768
H, W = 64, 64
NB, BS, MODES = 8, 96, 8
NCORES = 8
B_PER = B // NCORES          # 2 images per core
ROWS = B_PER * N_TOK         # 8192 rows per core
GROUP = 512                  # rows per group
NGROUPS = ROWS // GROUP      # 16
NSUB = GROUP // 128          # 4 subtiles of 128 rows
NCHUNK = C // 128            # 6 channel chunks
NCP = NCHUNK // 2            # 3 chunk pairs
NNS = C // 256               # 3 column splits of 256

DT = mybir.dt.float16
F8 = mybir.dt.float8e4
f32 = mybir.dt.float32
NPDT = np.float16

SC_PS = 32.0                 # psum scale (folded into m2h, undone in y evac)
SC_XLP = 4.0                 # xlpT evac divisor
SC_AB = SC_PS * SC_XLP       # block matrix pre-scale (128)
SC_XS = 4.0                  # xs evac divisor (xs8 = py/(SC_PS*SC_XS))
SC_W8 = SC_PS * SC_XS        # W' pre-scale (128)

# banded block-diagonal structure at 128-chunk granularity:
# ko -> contributing ki chunks; adjacent pairs run as fp8 DoubleRow
DR_LIST = [((0, 1), 0), ((0, 1), 1), ((1, 2), 2),
           ((3, 4), 3), ((3, 4), 4), ((4, 5), 5)]
SG_LIST = [(2, 1), (5, 4)]   # (ki, ko) leftovers as plain fp8 matmuls


def _filter_matrix():
    """M[w_in, w_out]: keep first MODES rfft modes along w (ortho norm)."""
    eye = np.eye(W)
    fw = np.fft.rfft(eye, axis=1, norm="ortho")
    fw[:, MODES:] = 0
    return np.fft.irfft(fw, n=W, axis=1, norm="ortho")  # symmetric


def _to_fp8(a):
    import ml_dtypes

    return a.astype(ml_dtypes.float8_e4m3fn)


def _build_consts(block_W, block_b, gates, W_out, b_out, rescale):
    g = 1.0 / (1.0 + np.exp(-gates.astype(np.float64)))
    m64 = _filter_matrix()

    m2l = np.zeros((128, 128))
    m2l[:64, :64] = m64
    m2l[64:, 64:] = m64
    m2h = SC_PS * (np.eye(128) - m2l)

    gmat = np.zeros((C, C))
    for b_ in range(NB):
        blk = g[b_] * block_W[b_].astype(np.float64)
        gmat[BS * b_ : BS * (b_ + 1), BS * b_ : BS * (b_ + 1)] = blk

    adr = np.zeros((128, len(DR_LIST), 2, 128))
    for idx, ((ka, kb), ko) in enumerate(DR_LIST):
        for t, ki in enumerate((ka, kb)):
            adr[:, idx, t, :] = SC_AB * gmat[
                128 * ki : 128 * (ki + 1), 128 * ko : 128 * (ko + 1)
            ]
    asg = np.zeros((128, len(SG_LIST), 128))
    for idx, (ki, ko) in enumerate(SG_LIST):
        asg[:, idx, :] = SC_AB * gmat[
            128 * ki : 128 * (ki + 1), 128 * ko : 128 * (ko + 1)
        ]

    wp = float(rescale) * W_out.astype(np.float64).T  # [C_in, C_out]
    w8c = np.zeros((128, NCP, NNS, 2, 256))
    for cp in range(NCP):
        for ns in range(NNS):
            for t in range(2):
                ci = 2 * cp + t
                w8c[:, cp, ns, t, :] = SC_W8 * wp[
                    128 * ci : 128 * (ci + 1), 256 * ns : 256 * (ns + 1)
                ]

    return (
        m2l.astype(NPDT),
        m2h.astype(NPDT),
        _to_fp8(np.eye(128)),
        _to_fp8(adr),
        _to_fp8(asg),
        _to_fp8(w8c),
    )


def _bias_correction(block_b, gates, W_out, b_out, rescale):
    """Batch-independent additive output term from the biases (zero for the
    standard inputs)."""
    if not (np.any(block_b) or np.any(b_out)):
        return None
    g = 1.0 / (1.0 + np.exp(-gates.astype(np.float64)))
    bias_freq = np.zeros((NB, BS, H, W // 2 + 1), dtype=np.complex128)
    bb = (g[:, None] * block_b.astype(np.float64))[:, :, None]
    bias_freq[:, :, :, :MODES] = np.broadcast_to(
        bb[:, :, :, None] * (1.0 + 1.0j), (NB, BS, H, MODES)
    )
    bias_img = np.fft.irfft2(bias_freq, s=(H, W), norm="ortho")  # [NB,BS,H,W]
    bias_img = bias_img.reshape(C, H * W).T  # [N_TOK, C]
    wfmat = float(rescale) * W_out.astype(np.float64).T + np.eye(C)
    corr = bias_img @ wfmat + float(rescale) * b_out.astype(np.float64)[None, :]
    return corr.astype(np.float32)  # [N_TOK, C]


def _build_kernel():
    nc = bacc.Bacc("TRN2", target_bir_lowering=False, debug=False, num_devices=NCORES)
    x_ext = nc.declare_dram_parameter("x", [ROWS, C], f32, isOutput=False)
    m2l_ext = nc.declare_dram_parameter("m2l", [128, 128], DT, isOutput=False)
    m2h_ext = nc.declare_dram_parameter("m2h", [128, 128], DT, isOutput=False)
    id8_ext = nc.declare_dram_parameter("id8", [128, 128], F8, isOutput=False)
    adr_ext = nc.declare_dram_parameter(
        "adr", [128, len(DR_LIST), 2, 128], F8, isOutput=False
    )
    asg_ext = nc.declare_dram_parameter(
        "asg", [128, len(SG_LIST), 128], F8, isOutput=False
    )
    w8c_ext = nc.declare_dram_parameter(
        "w8c", [128, NCP, NNS, 2, 256], F8, isOutput=False
    )
    out_ext = nc.declare_dram_parameter("out", [ROWS, C], DT, isOutput=True)

    DR = mybir.MatmulPerfMode.DoubleRow

    with tile.TileContext(nc) as tc:
        with (
            tc.tile_pool(name="const", bufs=1) as cpool,
            tc.tile_pool(name="io", bufs=4) as iopool,
            tc.tile_pool(name="work", bufs=2) as wpool,
            tc.tile_pool(name="ps_s", bufs=4, space="PSUM") as ps_s,
            tc.tile_pool(name="ps_y", bufs=2, space="PSUM") as ps_y,
        ):
            m2l = cpool.tile([128, 128], DT)
            nc.sync.dma_start(m2l[:], m2l_ext[:])
            m2h = cpool.tile([128, 128], DT)
            nc.sync.dma_start(m2h[:], m2h_ext[:])
            id8 = cpool.tile([128, 128], F8)
            nc.sync.dma_start(id8[:], id8_ext[:])
            adr = cpool.tile([128, len(DR_LIST), 2, 128], F8)
            nc.sync.dma_start(adr[:], adr_ext[:])
            asg = cpool.tile([128, len(SG_LIST), 128], F8)
            nc.sync.dma_start(asg[:], asg_ext[:])
            w8c = cpool.tile([128, NCP, NNS, 2, 256], F8)
            nc.sync.dma_start(w8c[:], w8c_ext[:])

            # last z writer per output chunk (for psum stop flags)
            last_ko = {ko: ("dr", i) for i, (_, ko) in enumerate(DR_LIST)}
            for i, (ki, ko) in enumerate(SG_LIST):
                last_ko[ko] = ("sg", i)

            for gidx in range(NGROUPS):
                r0 = gidx * GROUP
                # -- load + cast fp32 -> fp16 during DMA (SWDGE), per subtile
                xn = []
                for j in range(NSUB):
                    xj = iopool.tile([128, C], DT, tag=f"xn{j}")
                    nc.gpsimd.dma_start(
                        xj[:], x_ext[r0 + 128 * j : r0 + 128 * (j + 1), :]
                    )
                    xn.append(xj)

                # -- low-pass transposes: xlpT8[c, k, rows] = (xn_k^T @ M2)/4
                xlp8 = wpool.tile([128, NCHUNK, GROUP], F8, tag="xlp8")
                for k in range(NCHUNK):
                    pl = ps_s.tile([128, NSUB, 128], f32, tag="s")
                    for j in range(NSUB):
                        nc.tensor.matmul(
                            pl[:, j, :],
                            xn[j][:, 128 * k : 128 * (k + 1)],
                            m2l[:],
                            start=True,
                            stop=True,
                        )
                    nc.scalar.mul(xlp8[:, k, :], pl[:, :, :], 1.0 / SC_XLP)

                xst8 = wpool.tile([128, NSUB, NCHUNK, 128], F8, tag="xst8")
                for j in range(NSUB):
                    rows = slice(128 * j, 128 * (j + 1))
                    # -- py = 32*xs = hp + z
                    py = ps_y.tile([128, C], f32, tag="y")
                    nc.tensor.matmul(
                        py[:, 0:512], m2h[:], xn[j][:, 0:512],
                        start=True, stop=False,
                    )
                    nc.tensor.matmul(
                        py[:, 512:C], m2h[:], xn[j][:, 512:C],
                        start=True, stop=False,
                    )
                    for i, ((ka, _kb), ko) in enumerate(DR_LIST):
                        nc.tensor.matmul(
                            py[:, 128 * ko : 128 * (ko + 1)],
                            xlp8[:, ka : ka + 2, rows],
                            adr[:, i, :, :],
                            start=False,
                            stop=last_ko[ko] == ("dr", i),
                            perf_mode=DR,
                        )
                    for i, (ki, ko) in enumerate(SG_LIST):
                        nc.tensor.matmul(
                            py[:, 128 * ko : 128 * (ko + 1)],
                            xlp8[:, ki, rows],
                            asg[:, i, :],
                            start=False,
                            stop=last_ko[ko] == ("sg", i),
                        )
                    # -- row-space xs/4 in fp8 (for the transposes)
                    xs8 = wpool.tile([128, C], F8, tag=f"xs8{j}")
                    nc.vector.tensor_scalar_mul(
                        xs8[:], py[:], 1.0 / (SC_PS * SC_XS)
                    )
                    # -- fp8 transposes: xsT8[c, j, k, rows]
                    for h in range(2):
                        pt = ps_s.tile([128, NCHUNK // 2, 128], f32, tag="s")
                        for kk in range(NCHUNK // 2):
                            k = 3 * h + kk
                            nc.tensor.matmul(
                                pt[:, kk, :],
                                xs8[:, 128 * k : 128 * (k + 1)],
                                id8[:],
                                start=True,
                                stop=True,
                            )
                        if (j + h) % 2 == 0:
                            nc.scalar.copy(
                                xst8[:, j, 3 * h : 3 * (h + 1), :], pt[:, :, :]
                            )
                        else:
                            nc.vector.tensor_copy(
                                xst8[:, j, 3 * h : 3 * (h + 1), :], pt[:, :, :]
                            )
                    # -- projection accumulates straight onto py: py = 32*y
                    for cp in range(NCP):
                        for ns in range(NNS):
                            nc.tensor.matmul(
                                py[:, 256 * ns : 256 * (ns + 1)],
                                xst8[:, j, 2 * cp : 2 * cp + 2, :],
                                w8c[:, cp, ns, :, :],
                                start=False,
                                stop=(cp == NCP - 1),
                                perf_mode=DR,
                                skip_group_check=True,
                            )
                    # -- single mandatory evacuation: y = py/32, fp16 out
                    yj = iopool.tile([128, C], DT, tag=f"y{j}")
                    nc.vector.tensor_scalar_mul(yj[:], py[:], 1.0 / SC_PS)
                    nc.sync.dma_start(
                        out_ext[r0 + 128 * j : r0 + 128 * (j + 1), :], yj[:]
                    )
    nc.compile()
    return nc


_CACHED_NC = None


def _get_nc():
    global _CACHED_NC
    if _CACHED_NC is None:
        _CACHED_NC = _build_kernel()
    return _CACHED_NC


def _run(inputs, trace=False):
    x = np.ascontiguousarray(np.asarray(inputs["x"], dtype=np.float32))
    m2l, m2h, id8, adr, asg, w8c = _build_consts(
        np.asarray(inputs["block_W"], dtype=np.float32),
        np.asarray(inputs["block_b"], dtype=np.float32),
        np.asarray(inputs["gates"], dtype=np.float32),
        np.asarray(inputs["W_out"], dtype=np.float32),
        np.asarray(inputs["b_out"], dtype=np.float32),
        np.asarray(inputs["rescale"], dtype=np.float32),
    )
    corr = _bias_correction(
        np.asarray(inputs["block_b"], dtype=np.float32),
        np.asarray(inputs["gates"], dtype=np.float32),
        np.asarray(inputs["W_out"], dtype=np.float32),
        np.asarray(inputs["b_out"], dtype=np.float32),
        np.asarray(inputs["rescale"], dtype=np.float32),
    )

    nc = _get_nc()
    in_maps = []
    for i in range(NCORES):
        shard = x[i * B_PER : (i + 1) * B_PER].reshape(ROWS, C)
        in_maps.append(
            {"x": shard, "m2l": m2l, "m2h": m2h, "id8": id8,
             "adr": adr, "asg": asg, "w8c": w8c}
        )
    res = run_bass_kernel_spmd(
        nc, in_maps, core_ids=list(range(NCORES)), trace=trace
    )
    out = np.empty((B, N_TOK, C), dtype=np.float32)
    for i in range(NCORES):
        out[i * B_PER : (i + 1) * B_PER] = (
            np.asarray(res.results[i]["out"])
            .astype(np.float32)
            .reshape(B_PER, N_TOK, C)
        )
    if corr is not None:
        out += corr[None, :, :]
    return out, res.exec_time_ns


def kernel(**inputs) -> np.ndarray:
    out, _ = _run(inputs, trace=False)
    return out


# revision 11
# speedup vs baseline: 1.0995x; 1.0995x over previous
"""AFNO spectral attention kernel for 8 Trainium2 NeuronCores.

Math reduction (identical to the verified baseline):
  rfft2 -> first-8-W-mode block-diag channel matmul (x sigmoid(gate)) ->
  irfft2 -> residual -> out projection collapses to (H-axis FFTs cancel):

    xlp = x @ M2           M2 = w-axis low-pass (64x64, symmetric), per row
    xs  = (x - xlp) + xlp @ A       A = blockdiag(sigmoid(g_b) * W_b)
    y   = xs + xs @ W'              W' = rescale * W_out^T

Engine mapping per core (2 of 16 images, data-parallel over batch):
  Full-magnitude paths run fp16 on the PE; the small-coefficient channel
  matmuls (A ~0.01, W' ~0.002 entries) run fp8e4m3 with
  MatmulPerfMode.DoubleRow (two 128-channel k-tiles per matmul, 2x MACs).
  DR moving operands must be contiguous [K, 2, N] blocks (strided inner
  dims silently corrupt the second k-tile), so adr/w8c are stored
  pre-blocked per band / (chunk-pair, column-split).

  The single psum accumulator py carries 32*xs then 32*y:
    py[j]  = (32*Mh2)^T @ xn[j] + (xlp/4) @ (128*A)     [= 32*xs]
    py[j] += (xs/4) @ (128*W')                          [= 32*y]
  Power-of-2 scales keep every fp8 operand in e4m3's normal range (TRN
  flushes fp8 subnormals) and cancel exactly in the fp32 psum; the final
  1/32 rides the one mandatory psum->SBUF evacuation.

  Per group of 512 rows (4 subtiles of 128 rows = 2 h-rows x 64 w):
    xlpT8     = (xn[j][:,k]^T @ M2)/4       fp16 PE transpose, ACT evac fp8
    py[j]     = hp + z  (2 fp16 + 6 DR + 2 fp8 matmuls)
    xs8[j]    = py[j]/128                   DVE evac -> fp8 row-space xs/4
    xsT8[:,j] = xs8[j][:,k]^T @ I8          fp8 PE transposes, ACT evac
    py[j]    += proj (9 DR matmuls)
    y16[j]    = py[j]/32                    DVE evac -> fp16, DMA out
"""

import numpy as np

import concourse.mybir as mybir
import concourse.tile as tile
from concourse import bacc
from concourse.bass_utils import run_bass_kernel_spmd

B, N_TOK, C = 16, 4096, 768
H, W = 64, 64
NB, BS, MODES = 8, 96, 8
NCORES = 8
B_PER = B // NCORES          # 2 images per core
ROWS = B_PER * N_TOK         # 8192 rows per core
GROUP = 512                  # rows per group
NGROUPS = ROWS // GROUP      # 16
NSUB = GROUP // 128          # 4 subtiles of 128 rows
NCHUNK = C // 128            # 6 channel chunks
NCP = NCHUNK // 2            # 3 chunk pairs
NNS = C // 256               # 3 column splits of 256

DT = mybir.dt.float16
F8 = mybir.dt.float8e4
f32 = mybir.dt.float32
NPDT = np.float16

SC_PS = 32.0                 # psum scale (folded into m2h, undone in y evac)
SC_XLP = 4.0                 # xlpT evac divisor
SC_AB = SC_PS * SC_XLP       # block matrix pre-scale (128)
SC_XS = 4.0                  # xs evac divisor (xs8 = py/(SC_PS*SC_XS))
SC_W8 = SC_PS * SC_XS        # W' pre-scale (128)

# banded block-diagonal structure at 128-chunk granularity:
# ko -> contributing ki chunks; adjacent pairs run as fp8 DoubleRow
DR_LIST = [((0, 1), 0), ((0, 1), 1), ((1, 2), 2),
           ((3, 4), 3), ((3, 4), 4), ((4, 5), 5)]
SG_LIST = [(2, 1), (5, 4)]   # (ki, ko) leftovers as plain fp8 matmuls


def _filter_matrix():
    """M[w_in, w_out]: keep first MODES rfft modes along w (ortho norm)."""
    eye = np.eye(W)
    fw = np.fft.rfft(eye, axis=1, norm="ortho")
    fw[:, MODES:] = 0
    return np.fft.irfft(fw, n=W, axis=1, norm="ortho")  # symmetric


def _to_fp8(a):
    import ml_dtypes

    return a.astype(ml_dtypes.float8_e4m3fn)


def _build_consts(block_W, block_b, gates, W_out, b_out, rescale):
    g = 1.0 / (1.0 + np.exp(-gates.astype(np.float64)))
    m64 = _filter_matrix()

    m2l = np.zeros((128, 128))
    m2l[:64, :64] = m64
    m2l[64:, 64:] = m64
    m2h = SC_PS * (np.eye(128) - m2l)

    gmat = np.zeros((C, C))
    for b_ in range(NB):
        blk = g[b_] * block_W[b_].astype(np.float64)
        gmat[BS * b_ : BS * (b_ + 1), BS * b_ : BS * (b_ + 1)] = blk

    adr = np.zeros((128, len(DR_LIST), 2, 128))
    for idx, ((ka, kb), ko) in enumerate(DR_LIST):
        for t, ki in enumerate((ka, kb)):
            adr[:, idx, t, :] = SC_AB * gmat[
                128 * ki : 128 * (ki + 1), 128 * ko : 128 * (ko + 1)
            ]
    asg = np.zeros((128, len(SG_LIST), 128))
    for idx, (ki, ko) in enumerate(SG_LIST):
        asg[:, idx, :] = SC_AB * gmat[
            128 * ki : 128 * (ki + 1), 128 * ko : 128 * (ko + 1)
        ]

    wp = float(rescale) * W_out.astype(np.float64).T  # [C_in, C_out]
    w8c = np.zeros((128, NCP, NNS, 2, 256))
    for cp in range(NCP):
        for ns in range(NNS):
            for t in range(2):
                ci = 2 * cp + t
                w8c[:, cp, ns, t, :] = SC_W8 * wp[
                    128 * ci : 128 * (ci + 1), 256 * ns : 256 * (ns + 1)
                ]

    return (
        m2l.astype(NPDT),
        m2h.astype(NPDT),
        _to_fp8(np.eye(128)),
        _to_fp8(adr),
        _to_fp8(asg),
        _to_fp8(w8c),
    )


def _bias_correction(block_b, gates, W_out, b_out, rescale):
    """Batch-independent additive output term from the biases (zero for the
    standard inputs)."""
    if not (np.any(block_b) or np.any(b_out)):
        return None
    g = 1.0 / (1.0 + np.exp(-gates.astype(np.float64)))
    bias_freq = np.zeros((NB, BS, H, W // 2 + 1), dtype=np.complex128)
    bb = (g[:, None] * block_b.astype(np.float64))[:, :, None]
    bias_freq[:, :, :, :MODES] = np.broadcast_to(
        bb[:, :, :, None] * (1.0 + 1.0j), (NB, BS, H, MODES)
    )
    bias_img = np.fft.irfft2(bias_freq, s=(H, W), norm="ortho")  # [NB,BS,H,W]
    bias_img = bias_img.reshape(C, H * W).T  # [N_TOK, C]
    wfmat = float(rescale) * W_out.astype(np.float64).T + np.eye(C)
    corr = bias_img @ wfmat + float(rescale) * b_out.astype(np.float64)[None, :]
    return corr.astype(np.float32)  # [N_TOK, C]


def _build_kernel():
    nc = bacc.Bacc("TRN2", target_bir_lowering=False, debug=False, num_devices=NCORES)
    x_ext = nc.declare_dram_parameter("x", [ROWS, C], f32, isOutput=False)
    m2l_ext = nc.declare_dram_parameter("m2l", [128, 128], DT, isOutput=False)
    m2h_ext = nc.declare_dram_parameter("m2h", [128, 128], DT, isOutput=False)
    id8_ext = nc.declare_dram_parameter("id8", [128, 128], F8, isOutput=False)
    adr_ext = nc.declare_dram_parameter(
        "adr", [128, len(DR_LIST), 2, 128], F8, isOutput=False
    )
    asg_ext = nc.declare_dram_parameter(
        "asg", [128, len(SG_LIST), 128], F8, isOutput=False
    )
    w8c_ext = nc.declare_dram_parameter(
        "w8c", [128, NCP, NNS, 2, 256], F8, isOutput=False
    )
    out_ext = nc.declare_dram_parameter("out", [ROWS, C], DT, isOutput=True)

    DR = mybir.MatmulPerfMode.DoubleRow

    with tile.TileContext(nc) as tc:
        with (
            tc.tile_pool(name="const", bufs=1) as cpool,
            tc.tile_pool(name="io", bufs=3) as iopool,
            tc.tile_pool(name="work", bufs=2) as wpool,
            tc.tile_pool(name="ps_l", bufs=2, space="PSUM") as ps_l,
            tc.tile_pool(name="ps_t", bufs=2, space="PSUM") as ps_t,
            tc.tile_pool(name="ps_y", bufs=2, space="PSUM") as ps_y,
        ):
            m2l = cpool.tile([128, 128], DT)
            nc.sync.dma_start(m2l[:], m2l_ext[:])
            m2h = cpool.tile([128, 128], DT)
            nc.sync.dma_start(m2h[:], m2h_ext[:])
            id8 = cpool.tile([128, 128], F8)
            nc.sync.dma_start(id8[:], id8_ext[:])
            adr = cpool.tile([128, len(DR_LIST), 2, 128], F8)
            nc.sync.dma_start(adr[:], adr_ext[:])
            asg = cpool.tile([128, len(SG_LIST), 128], F8)
            nc.sync.dma_start(asg[:], asg_ext[:])
            w8c = cpool.tile([128, NCP, NNS, 2, 256], F8)
            nc.sync.dma_start(w8c[:], w8c_ext[:])

            # last z writer per output chunk (for psum stop flags)
            last_ko = {ko: ("dr", i) for i, (_, ko) in enumerate(DR_LIST)}
            for i, (ki, ko) in enumerate(SG_LIST):
                last_ko[ko] = ("sg", i)

            for gidx in range(NGROUPS):
                r0 = gidx * GROUP
                # -- load + cast fp32 -> fp16 during DMA (SWDGE), per subtile
                xn = []
                for j in range(NSUB):
                    xj = iopool.tile([128, C], DT, tag=f"xn{j}")
                    nc.gpsimd.dma_start(
                        xj[:], x_ext[r0 + 128 * j : r0 + 128 * (j + 1), :]
                    )
                    xn.append(xj)

                # -- low-pass transposes: xlpT8[c, k, rows] = (xn_k^T @ M2)/4
                xlp8 = wpool.tile([128, NCHUNK, GROUP], F8, tag="xlp8")
                for k in range(NCHUNK):
                    pl = ps_l.tile([128, NSUB, 128], f32, tag="l")
                    for j in range(NSUB):
                        nc.tensor.matmul(
                            pl[:, j, :],
                            xn[j][:, 128 * k : 128 * (k + 1)],
                            m2l[:],
                            start=True,
                            stop=True,
                        )
                    nc.scalar.mul(xlp8[:, k, :], pl[:, :, :], 1.0 / SC_XLP)

                xst8 = wpool.tile([128, NSUB, NCHUNK, 128], F8, tag="xst8")
                for j in range(NSUB):
                    rows = slice(128 * j, 128 * (j + 1))
                    # -- py = 32*xs = hp + z
                    py = ps_y.tile([128, C], f32, tag="y")
                    nc.tensor.matmul(
                        py[:, 0:512], m2h[:], xn[j][:, 0:512],
                        start=True, stop=False,
                    )
                    nc.tensor.matmul(
                        py[:, 512:C], m2h[:], xn[j][:, 512:C],
                        start=True, stop=False,
                    )
                    for i, ((ka, _kb), ko) in enumerate(DR_LIST):
                        nc.tensor.matmul(
                            py[:, 128 * ko : 128 * (ko + 1)],
                            xlp8[:, ka : ka + 2, rows],
                            adr[:, i, :, :],
                            start=False,
                            stop=last_ko[ko] == ("dr", i),
                            perf_mode=DR,
                        )
                    for i, (ki, ko) in enumerate(SG_LIST):
                        nc.tensor.matmul(
                            py[:, 128 * ko : 128 * (ko + 1)],
                            xlp8[:, ki, rows],
                            asg[:, i, :],
                            start=False,
                            stop=last_ko[ko] == ("sg", i),
                        )
                    # -- row-space xs/4 in fp8 (for the transposes)
                    xs8 = wpool.tile([128, C], F8, tag=f"xs8{j}")
                    nc.vector.tensor_scalar_mul(
                        xs8[:], py[:], 1.0 / (SC_PS * SC_XS)
                    )
                    # -- fp8 transposes: xsT8[c, j, k, rows]
                    for h in range(2):
                        pt = ps_t.tile([128, NCHUNK // 2, 128], f32, tag="t")
                        for kk in range(NCHUNK // 2):
                            k = 3 * h + kk
                            nc.tensor.matmul(
                                pt[:, kk, :],
                                xs8[:, 128 * k : 128 * (k + 1)],
                                id8[:],
                                start=True,
                                stop=True,
                            )
                        if (j + h) % 2 == 0:
                            nc.scalar.copy(
                                xst8[:, j, 3 * h : 3 * (h + 1), :], pt[:, :, :]
                            )
                        else:
                            nc.vector.tensor_copy(
                                xst8[:, j, 3 * h : 3 * (h + 1), :], pt[:, :, :]
                            )
                    # -- projection accumulates straight onto py: py = 32*y
                    for cp in range(NCP):
                        for ns in range(NNS):
                            nc.tensor.matmul(
                                py[:, 256 * ns : 256 * (ns + 1)],
                                xst8[:, j, 2 * cp : 2 * cp + 2, :],
                                w8c[:, cp, ns, :, :],
                                start=False,
                                stop=(cp == NCP - 1),
                                perf_mode=DR,
                                skip_group_check=True,
                            )
                    # -- single mandatory evacuation: y = py/32, fp16 out
                    yj = iopool.tile([128, C], DT, tag=f"y{j}")
                    nc.vector.tensor_scalar_mul(yj[:], py[:], 1.0 / SC_PS)
                    nc.sync.dma_start(
                        out_ext[r0 + 128 * j : r0 + 128 * (j + 1), :], yj[:]
                    )
    nc.compile()
    return nc


_CACHED_NC = None


def _get_nc():
    global _CACHED_NC
    if _CACHED_NC is None:
        _CACHED_NC = _build_kernel()
    return _CACHED_NC


def _run(inputs, trace=False):
    x = np.ascontiguousarray(np.asarray(inputs["x"], dtype=np.float32))
    m2l, m2h, id8, adr, asg, w8c = _build_consts(
        np.asarray(inputs["block_W"], dtype=np.float32),
        np.asarray(inputs["block_b"], dtype=np.float32),
        np.asarray(inputs["gates"], dtype=np.float32),
        np.asarray(inputs["W_out"], dtype=np.float32),
        np.asarray(inputs["b_out"], dtype=np.float32),
        np.asarray(inputs["rescale"], dtype=np.float32),
    )
    corr = _bias_correction(
        np.asarray(inputs["block_b"], dtype=np.float32),
        np.asarray(inputs["gates"], dtype=np.float32),
        np.asarray(inputs["W_out"], dtype=np.float32),
        np.asarray(inputs["b_out"], dtype=np.float32),
        np.asarray(inputs["rescale"], dtype=np.float32),
    )

    nc = _get_nc()
    in_maps = []
    for i in range(NCORES):
        shard = x[i * B_PER : (i + 1) * B_PER].reshape(ROWS, C)
        in_maps.append(
            {"x": shard, "m2l": m2l, "m2h": m2h, "id8": id8,
             "adr": adr, "asg": asg, "w8c": w8c}
        )
    res = run_bass_kernel_spmd(
        nc, in_maps, core_ids=list(range(NCORES)), trace=trace
    )
    out = np.empty((B, N_TOK, C), dtype=np.float32)
    for i in range(NCORES):
        out[i * B_PER : (i + 1) * B_PER] = (
            np.asarray(res.results[i]["out"])
            .astype(np.float32)
            .reshape(B_PER, N_TOK, C)
        )
    if corr is not None:
        out += corr[None, :, :]
    return out, res.exec_time_ns


def kernel(**inputs) -> np.ndarray:
    out, _ = _run(inputs, trace=False)
    return out


# revision 12
# speedup vs baseline: 1.1034x; 1.0035x over previous
"""AFNO spectral attention kernel for 8 Trainium2 NeuronCores.

Math reduction (identical to the verified baseline):
  rfft2 -> first-8-W-mode block-diag channel matmul (x sigmoid(gate)) ->
  irfft2 -> residual -> out projection collapses to (H-axis FFTs cancel):

    xlp = x @ M2           M2 = w-axis low-pass (64x64, symmetric), per row
    xs  = (x - xlp) + xlp @ A       A = blockdiag(sigmoid(g_b) * W_b)
    y   = xs + xs @ W'              W' = rescale * W_out^T

Engine mapping per core (2 of 16 images, data-parallel over batch):
  Full-magnitude paths run fp16 on the PE; the small-coefficient channel
  matmuls (A ~0.01, W' ~0.002 entries) run fp8e4m3 with
  MatmulPerfMode.DoubleRow (two 128-channel k-tiles per matmul, 2x MACs).
  DR moving operands must be contiguous [K, 2, N] blocks (strided inner
  dims silently corrupt the second k-tile), so adr/w8c are stored
  pre-blocked per band / (chunk-pair, column-split).

  The single psum accumulator py carries 32*xs then 32*y:
    py[j]  = (32*Mh2)^T @ xn[j] + (xlp/4) @ (128*A)     [= 32*xs]
    py[j] += (xs/4) @ (128*W')                          [= 32*y]
  Power-of-2 scales keep every fp8 operand in e4m3's normal range (TRN
  flushes fp8 subnormals) and cancel exactly in the fp32 psum; the final
  1/32 rides the one mandatory psum->SBUF evacuation.

  Per group of 512 rows (4 subtiles of 128 rows = 2 h-rows x 64 w):
    xlpT8     = (xn[j][:,k]^T @ M2)/4       fp16 PE transpose, ACT evac fp8
    py[j]     = hp + z  (2 fp16 + 6 DR + 2 fp8 matmuls)
    xs8[j]    = py[j]/128                   DVE evac -> fp8 row-space xs/4
    xsT8[:,j] = xs8[j][:,k]^T @ I8          fp8 PE transposes, ACT evac
    py[j]    += proj (9 DR matmuls)
    y16[j]    = py[j]/32                    DVE evac -> fp16, DMA out
"""

import numpy as np

import concourse.mybir as mybir
import concourse.tile as tile
from concourse import bacc
from concourse.bass_utils import run_bass_kernel_spmd

B, N_TOK, C = 16, 4096, 768
H, W = 64, 64
NB, BS, MODES = 8, 96, 8
NCORES = 8
B_PER = B // NCORES          # 2 images per core
ROWS = B_PER * N_TOK         # 8192 rows per core
GROUP = 512                  # rows per group
NGROUPS = ROWS // GROUP      # 16
NSUB = GROUP // 128          # 4 subtiles of 128 rows
NCHUNK = C // 128            # 6 channel chunks
NCP = NCHUNK // 2            # 3 chunk pairs
NNS = C // 256               # 3 column splits of 256

DT = mybir.dt.float16
F8 = mybir.dt.float8e4
f32 = mybir.dt.float32
NPDT = np.float16

SC_PS = 32.0                 # psum scale (folded into m2h, undone in y evac)
SC_XLP = 4.0                 # xlpT evac divisor
SC_AB = SC_PS * SC_XLP       # block matrix pre-scale (128)
SC_XS = 4.0                  # xs evac divisor (xs8 = py/(SC_PS*SC_XS))
SC_W8 = SC_PS * SC_XS        # W' pre-scale (128)

# banded block-diagonal structure at 128-chunk granularity:
# ko -> contributing ki chunks; adjacent pairs run as fp8 DoubleRow
DR_LIST = [((0, 1), 0), ((0, 1), 1), ((1, 2), 2),
           ((3, 4), 3), ((3, 4), 4), ((4, 5), 5)]
SG_LIST = [(2, 1), (5, 4)]   # (ki, ko) leftovers as plain fp8 matmuls


def _filter_matrix():
    """M[w_in, w_out]: keep first MODES rfft modes along w (ortho norm)."""
    eye = np.eye(W)
    fw = np.fft.rfft(eye, axis=1, norm="ortho")
    fw[:, MODES:] = 0
    return np.fft.irfft(fw, n=W, axis=1, norm="ortho")  # symmetric


def _to_fp8(a):
    import ml_dtypes

    return a.astype(ml_dtypes.float8_e4m3fn)


def _build_consts(block_W, block_b, gates, W_out, b_out, rescale):
    g = 1.0 / (1.0 + np.exp(-gates.astype(np.float64)))
    m64 = _filter_matrix()

    m2l = np.zeros((128, 128))
    m2l[:64, :64] = m64
    m2l[64:, 64:] = m64
    m2h = SC_PS * (np.eye(128) - m2l)

    gmat = np.zeros((C, C))
    for b_ in range(NB):
        blk = g[b_] * block_W[b_].astype(np.float64)
        gmat[BS * b_ : BS * (b_ + 1), BS * b_ : BS * (b_ + 1)] = blk

    adr = np.zeros((128, len(DR_LIST), 2, 128))
    for idx, ((ka, kb), ko) in enumerate(DR_LIST):
        for t, ki in enumerate((ka, kb)):
            adr[:, idx, t, :] = SC_AB * gmat[
                128 * ki : 128 * (ki + 1), 128 * ko : 128 * (ko + 1)
            ]
    asg = np.zeros((128, len(SG_LIST), 128))
    for idx, (ki, ko) in enumerate(SG_LIST):
        asg[:, idx, :] = SC_AB * gmat[
            128 * ki : 128 * (ki + 1), 128 * ko : 128 * (ko + 1)
        ]

    wp = float(rescale) * W_out.astype(np.float64).T  # [C_in, C_out]
    w8c = np.zeros((128, NCP, NNS, 2, 256))
    for cp in range(NCP):
        for ns in range(NNS):
            for t in range(2):
                ci = 2 * cp + t
                w8c[:, cp, ns, t, :] = SC_W8 * wp[
                    128 * ci : 128 * (ci + 1), 256 * ns : 256 * (ns + 1)
                ]

    return (
        m2l.astype(NPDT),
        m2h.astype(NPDT),
        _to_fp8(np.eye(128)),
        _to_fp8(adr),
        _to_fp8(asg),
        _to_fp8(w8c),
    )


def _bias_correction(block_b, gates, W_out, b_out, rescale):
    """Batch-independent additive output term from the biases (zero for the
    standard inputs)."""
    if not (np.any(block_b) or np.any(b_out)):
        return None
    g = 1.0 / (1.0 + np.exp(-gates.astype(np.float64)))
    bias_freq = np.zeros((NB, BS, H, W // 2 + 1), dtype=np.complex128)
    bb = (g[:, None] * block_b.astype(np.float64))[:, :, None]
    bias_freq[:, :, :, :MODES] = np.broadcast_to(
        bb[:, :, :, None] * (1.0 + 1.0j), (NB, BS, H, MODES)
    )
    bias_img = np.fft.irfft2(bias_freq, s=(H, W), norm="ortho")  # [NB,BS,H,W]
    bias_img = bias_img.reshape(C, H * W).T  # [N_TOK, C]
    wfmat = float(rescale) * W_out.astype(np.float64).T + np.eye(C)
    corr = bias_img @ wfmat + float(rescale) * b_out.astype(np.float64)[None, :]
    return corr.astype(np.float32)  # [N_TOK, C]


def _build_kernel():
    nc = bacc.Bacc("TRN2", target_bir_lowering=False, debug=False, num_devices=NCORES)
    x_ext = nc.declare_dram_parameter("x", [ROWS, C], f32, isOutput=False)
    m2l_ext = nc.declare_dram_parameter("m2l", [128, 128], DT, isOutput=False)
    m2h_ext = nc.declare_dram_parameter("m2h", [128, 128], DT, isOutput=False)
    id8_ext = nc.declare_dram_parameter("id8", [128, 128], F8, isOutput=False)
    adr_ext = nc.declare_dram_parameter(
        "adr", [128, len(DR_LIST), 2, 128], F8, isOutput=False
    )
    asg_ext = nc.declare_dram_parameter(
        "asg", [128, len(SG_LIST), 128], F8, isOutput=False
    )
    w8c_ext = nc.declare_dram_parameter(
        "w8c", [128, NCP, NNS, 2, 256], F8, isOutput=False
    )
    out_ext = nc.declare_dram_parameter("out", [ROWS, C], DT, isOutput=True)

    DR = mybir.MatmulPerfMode.DoubleRow

    with tile.TileContext(nc) as tc:
        with (
            tc.tile_pool(name="const", bufs=1) as cpool,
            tc.tile_pool(name="io", bufs=3) as iopool,
            tc.tile_pool(name="work", bufs=2) as wpool,
            tc.tile_pool(name="ps_l", bufs=2, space="PSUM") as ps_l,
            tc.tile_pool(name="ps_t", bufs=2, space="PSUM") as ps_t,
            tc.tile_pool(name="ps_y", bufs=2, space="PSUM") as ps_y,
        ):
            m2l = cpool.tile([128, 128], DT)
            nc.sync.dma_start(m2l[:], m2l_ext[:])
            m2h = cpool.tile([128, 128], DT)
            nc.sync.dma_start(m2h[:], m2h_ext[:])
            id8 = cpool.tile([128, 128], F8)
            nc.sync.dma_start(id8[:], id8_ext[:])
            adr = cpool.tile([128, len(DR_LIST), 2, 128], F8)
            nc.sync.dma_start(adr[:], adr_ext[:])
            asg = cpool.tile([128, len(SG_LIST), 128], F8)
            nc.sync.dma_start(asg[:], asg_ext[:])
            w8c = cpool.tile([128, NCP, NNS, 2, 256], F8)
            nc.sync.dma_start(w8c[:], w8c_ext[:])

            # last z writer per output chunk (for psum stop flags)
            last_ko = {ko: ("dr", i) for i, (_, ko) in enumerate(DR_LIST)}
            for i, (ki, ko) in enumerate(SG_LIST):
                last_ko[ko] = ("sg", i)

            def emit_front(gidx):
                """DMA loads + low-pass transposes for group gidx."""
                r0 = gidx * GROUP
                xn = []
                for j in range(NSUB):
                    xj = iopool.tile([128, C], DT, tag=f"xn{j}")
                    nc.gpsimd.dma_start(
                        xj[:], x_ext[r0 + 128 * j : r0 + 128 * (j + 1), :]
                    )
                    xn.append(xj)
                xlp8 = wpool.tile([128, NCHUNK, GROUP], F8, tag="xlp8")
                for k in range(NCHUNK):
                    pl = ps_l.tile([128, NSUB, 128], f32, tag="l")
                    for j in range(NSUB):
                        nc.tensor.matmul(
                            pl[:, j, :],
                            xn[j][:, 128 * k : 128 * (k + 1)],
                            m2l[:],
                            start=True,
                            stop=True,
                        )
                    nc.scalar.mul(xlp8[:, k, :], pl[:, :, :], 1.0 / SC_XLP)
                return xn, xlp8

            def emit_body(gidx, xn, xlp8):
                r0 = gidx * GROUP
                xst8 = wpool.tile([128, NSUB, NCHUNK, 128], F8, tag="xst8")
                for j in range(NSUB):
                    rows = slice(128 * j, 128 * (j + 1))
                    # -- py = 32*xs = hp + z
                    py = ps_y.tile([128, C], f32, tag="y")
                    nc.tensor.matmul(
                        py[:, 0:512], m2h[:], xn[j][:, 0:512],
                        start=True, stop=False,
                    )
                    nc.tensor.matmul(
                        py[:, 512:C], m2h[:], xn[j][:, 512:C],
                        start=True, stop=False,
                    )
                    for i, ((ka, _kb), ko) in enumerate(DR_LIST):
                        nc.tensor.matmul(
                            py[:, 128 * ko : 128 * (ko + 1)],
                            xlp8[:, ka : ka + 2, rows],
                            adr[:, i, :, :],
                            start=False,
                            stop=last_ko[ko] == ("dr", i),
                            perf_mode=DR,
                        )
                    for i, (ki, ko) in enumerate(SG_LIST):
                        nc.tensor.matmul(
                            py[:, 128 * ko : 128 * (ko + 1)],
                            xlp8[:, ki, rows],
                            asg[:, i, :],
                            start=False,
                            stop=last_ko[ko] == ("sg", i),
                        )
                    # -- row-space xs/4 in fp8 (for the transposes)
                    xs8 = wpool.tile([128, C], F8, tag=f"xs8{j}")
                    nc.vector.tensor_scalar_mul(
                        xs8[:], py[:], 1.0 / (SC_PS * SC_XS)
                    )
                    # -- fp8 transposes: xsT8[c, j, k, rows]
                    for h in range(2):
                        pt = ps_t.tile([128, NCHUNK // 2, 128], f32, tag="t")
                        for kk in range(NCHUNK // 2):
                            k = 3 * h + kk
                            nc.tensor.matmul(
                                pt[:, kk, :],
                                xs8[:, 128 * k : 128 * (k + 1)],
                                id8[:],
                                start=True,
                                stop=True,
                            )
                        if (j + h) % 2 == 0:
                            nc.scalar.copy(
                                xst8[:, j, 3 * h : 3 * (h + 1), :], pt[:, :, :]
                            )
                        else:
                            nc.vector.tensor_copy(
                                xst8[:, j, 3 * h : 3 * (h + 1), :], pt[:, :, :]
                            )
                    # -- projection accumulates straight onto py: py = 32*y
                    for cp in range(NCP):
                        for ns in range(NNS):
                            nc.tensor.matmul(
                                py[:, 256 * ns : 256 * (ns + 1)],
                                xst8[:, j, 2 * cp : 2 * cp + 2, :],
                                w8c[:, cp, ns, :, :],
                                start=False,
                                stop=(cp == NCP - 1),
                                perf_mode=DR,
                                skip_group_check=True,
                            )
                    # -- single mandatory evacuation: y = py/32, fp16 out
                    yj = iopool.tile([128, C], DT, tag=f"y{j}")
                    nc.vector.tensor_scalar_mul(yj[:], py[:], 1.0 / SC_PS)
                    nc.sync.dma_start(
                        out_ext[r0 + 128 * j : r0 + 128 * (j + 1), :], yj[:]
                    )

            # software pipeline: front(g+1) is emitted before body(g)
            front = emit_front(0)
            for gidx in range(NGROUPS):
                nxt = emit_front(gidx + 1) if gidx + 1 < NGROUPS else None
                emit_body(gidx, *front)
                front = nxt
    nc.compile()
    return nc


_CACHED_NC = None


def _get_nc():
    global _CACHED_NC
    if _CACHED_NC is None:
        _CACHED_NC = _build_kernel()
    return _CACHED_NC


def _run(inputs, trace=False):
    x = np.ascontiguousarray(np.asarray(inputs["x"], dtype=np.float32))
    m2l, m2h, id8, adr, asg, w8c = _build_consts(
        np.asarray(inputs["block_W"], dtype=np.float32),
        np.asarray(inputs["block_b"], dtype=np.float32),
        np.asarray(inputs["gates"], dtype=np.float32),
        np.asarray(inputs["W_out"], dtype=np.float32),
        np.asarray(inputs["b_out"], dtype=np.float32),
        np.asarray(inputs["rescale"], dtype=np.float32),
    )
    corr = _bias_correction(
        np.asarray(inputs["block_b"], dtype=np.float32),
        np.asarray(inputs["gates"], dtype=np.float32),
        np.asarray(inputs["W_out"], dtype=np.float32),
        np.asarray(inputs["b_out"], dtype=np.float32),
        np.asarray(inputs["rescale"], dtype=np.float32),
    )

    nc = _get_nc()
    in_maps = []
    for i in range(NCORES):
        shard = x[i * B_PER : (i + 1) * B_PER].reshape(ROWS, C)
        in_maps.append(
            {"x": shard, "m2l": m2l, "m2h": m2h, "id8": id8,
             "adr": adr, "asg": asg, "w8c": w8c}
        )
    res = run_bass_kernel_spmd(
        nc, in_maps, core_ids=list(range(NCORES)), trace=trace
    )
    out = np.empty((B, N_TOK, C), dtype=np.float32)
    for i in range(NCORES):
        out[i * B_PER : (i + 1) * B_PER] = (
            np.asarray(res.results[i]["out"])
            .astype(np.float32)
            .reshape(B_PER, N_TOK, C)
        )
    if corr is not None:
        out += corr[None, :, :]
    return out, res.exec_time_ns


def kernel(**inputs) -> np.ndarray:
    out, _ = _run(inputs, trace=False)
    return out
